# revision 38
# baseline (speedup 1.0000x reference)
"""Trainium2 Bass kernel for nn_BinTreeNetwork (binary-tree MLP expansion).

Strategy
--------
The reference is a 21-level binary-tree expansion ending at a (2,)^21 x 32
fp32 output (256 MB). Everything is linear; in flat row terms each level
doubles the rows via out'[r'] = out[r' mod M] + C[r'], C = res @ Wo_i.T,
so the final output row r is

  out[r] = o_L0[r mod 2^L0] + sum_{i=L0..20} C_i[r mod 2^(i+1)] + bias.

Row-index bits make a mod-8 row sharding communication-free: core q owns
rows ≡ q (mod 8). The host computes the tiny exact state path (L/R, 2
floats per row) through all levels and the o-accumulator up to level
L0=17, then ships per core the pre-tiled res-plane stack of levels
17..20 ([128, 16384] fp16, with the stationary weights packed into the
leading 128 columns so one descriptor unblocks chunk 0) and o17 (the
level-17 o accumulator + out_bias, packed [128, 4096] fp16).

The stack is interleaved into four 32-partition blocks so the K=32
matmuls run as four concurrent tile_position row-groups. No PE warmup
on purpose: a HAM-warm (2.4 GHz) PE downclocks DVE/Act/Pool ~20% and
this kernel is vector/DMA-bound.

Packed layouts ("mod-4 stacked planes"): o/out tiles are [128, M/4]:
partition 32*(row%4)+plane, column row//4. All level wraps become
column slices, every engine op runs at full width, and all DMAs are
contiguous.

DMA model (measured): queues execute descriptors serially; HWDGE
read-descriptors bog down near ~80 GB/s while SWDGE sustains ~190 GB/s
on ~1 MB descriptors, and write throughput scales with per-partition
packet size (4 KB packets ~140 GB/s/queue, 8 KB ~220). Hence: the bulk
of the input streams on the SWDGE queue as few large descriptors in
deadline order, the startup-critical slices (first stack columns + o's
first half) take the first-descriptor slot of each queue, and the out
stream is staged in 4-chunk [128, 8192] mega tiles written as two
[128, 4096] half-descriptors on the SP and Act queues simultaneously
(8 KB packets, both queues busy every mega). Chunks are processed
evens-then-odds per 8-block (the out tensor is laid out in processing
order and re-permuted on the host) so o's second half is not needed
until position 4, removing it from the startup critical path.

All DRAM traffic is fp16 (~22 MiB/core): in ~5.2 + out 16.78. PSUM
accumulation stays fp32; fp16 rounding costs ~3e-4 relative error
(gate is 2e-2). The PSUM->fp16+o stage is split two ways from the
measured cost model — DVE fused tensor_tensor from PSUM on [0:xv],
Act copy + Pool add on [xv:2048] — balancing the three engines at a
~1.65 us/chunk cadence. Mega out-DMAs are issued one chunk late so
their sem-waits never block either HWDGE FIFO; the final mega drains
in quarters with the last quarter split across both queues.
"""
import os
import numpy as np
from contextlib import ExitStack

import concourse.bass as bass
import concourse.bacc as bacc
import concourse.mybir as mybir
import concourse.tile as tile
from concourse.bass_utils import run_bass_kernel_spmd

T = 21
L0 = 17
CHUNK = 2048
N = 1 << (T - 5)          # 65536 final packed cols per core
NT = N // 4               # 16384 "tall" stack cols
if os.environ.get("BT_DT", "f16") == "bf16":
    import ml_dtypes
    F16 = mybir.dt.bfloat16
    NP16 = ml_dtypes.bfloat16
else:
    F16 = mybir.dt.float16
    NP16 = np.float16
F32 = mybir.dt.float32

_CACHE = {}


# ---------------- host-side exact precompute ----------------

def _host_precompute(inputs):
    x = inputs["x"].astype(np.float32)
    L = (x @ inputs["in_left_layer"].T + inputs["in_left_bias"]).reshape(1, 2).astype(np.float32)
    R = (x @ inputs["in_right_layer"].T + inputs["in_right_bias"]).reshape(1, 2).astype(np.float32)
    out = (x @ inputs["out_layer0"].T).reshape(1, 32).astype(np.float32)
    res_levels = []
    o_L0 = None
    for i in range(T):
        M = L.shape[0]
        if i == L0:
            o_L0 = out
        if M == 1:
            res = np.array([[L[0, 0], R[0, 0]], [L[0, 1], R[0, 1]]], np.float32)
        else:
            res = np.concatenate([L[: M // 2], R[: M // 2], L[M // 2 :], R[M // 2 :]], axis=0)
        if i >= L0:
            res_levels.append(res)
        else:
            C = res @ inputs["out_layers"][i].T
            out = np.concatenate([out + C[:M], out + C[M:]], axis=0)
        if i < T - 1:  # last level's L/R states are unused
            L = res @ inputs["tree_left_layers"][i].T + inputs["tree_left_biases"][i]
            R = res @ inputs["tree_right_layers"][i].T + inputs["tree_right_biases"][i]
    o_L0 = o_L0 + inputs["out_bias"].astype(np.float32)[None, :]
    return o_L0, res_levels


def _pack_o_mod4(o_rows):
    M = o_rows.shape[0]
    return np.ascontiguousarray(
        o_rows.reshape(M // 4, 4, 32).transpose(1, 2, 0).reshape(128, M // 4))


def _unpack_o_mod4(t):
    Mc = t.shape[1]
    return np.ascontiguousarray(
        t.reshape(4, 32, Mc).transpose(2, 0, 1).reshape(4 * Mc, 32))


def _pack_res8(res):
    m2 = res.shape[0]
    cols = m2 // 4
    return np.ascontiguousarray(
        res.reshape(cols, 4, 2).transpose(1, 2, 0).reshape(8, cols))


def _make_lhsT(Wo):
    t = np.zeros((8, 128), np.float32)
    for b in range(4):
        for f in range(2):
            t[2 * b + f, 32 * b: 32 * (b + 1)] = Wo[:, f]
    return t


# ---------------- device program ----------------

def _build_nc():
    nlev = T - L0
    K = 8 * nlev
    assert K == 32, "row-group interleave assumes K=32 (L0=17)"
    ocols = 1 << (L0 - 5)

    nc = bacc.Bacc("TRN2", target_bir_lowering=False, debug=False,
                   enable_asserts=True, num_devices=8)

    o_d = nc.dram_tensor("o_init", [128, ocols], F16, kind="ExternalInput").ap()
    # wc (128 cols) + pieces 0-3 (8192 cols) in one image so a single
    # leading descriptor unblocks chunk0's ldweights+matmuls
    p03_d = nc.dram_tensor("p03", [128, 128 + 8192], F16, kind="ExternalInput").ap()
    p47_d = nc.dram_tensor("p47", [128, 8192], F16, kind="ExternalInput").ap()
    out_d = nc.dram_tensor("out", [128, N], F16, kind="ExternalOutput").ap()

    with tile.TileContext(nc, trace_sim=False) as tc:
        ctx = ExitStack()
        with ctx:
            const_pool = ctx.enter_context(tc.tile_pool(name="consts", bufs=1))
            outc_pool = ctx.enter_context(tc.tile_pool(name="outc", bufs=4))
            tmp_pool = ctx.enter_context(tc.tile_pool(name="tmp", bufs=4))
            psum_pool = ctx.enter_context(tc.tile_pool(name="ps", bufs=2, space="PSUM"))

            stk = const_pool.tile([128, 128 + NT], F16, name="stk")
            wc_sb = stk  # stationary weights live in stk cols [0:128]
            o_sb = const_pool.tile([128, ocols], F16, name="o_sb")

            # --- inputs. Queues execute descriptors serially: HWDGE reads
            # bog down near ~80 GB/s while SWDGE sustains ~190 GB/s on ~1 MB
            # descriptors. The startup-critical slices (wc+first stack cols,
            # o's first half) take the first-descriptor slot of each queue;
            # the bulk streams on the SWDGE queue as few large descriptors
            # in consumption-deadline order. o's second half is first needed
            # at position 4 (evens-then-odds chunk order), pieces 4-7 from
            # position 16.
            nc.sync.dma_start(out=stk[:, 0:2176], in_=p03_d[:, 0:2176])
            nc.scalar.dma_start(out=o_sb[:, 0:1024], in_=o_d[:, 0:1024])
            nc.gpsimd.dma_start(out=o_sb[:, 1024:2048], in_=o_d[:, 1024:2048])
            nc.gpsimd.dma_start(out=stk[:, 2176:4224], in_=p03_d[:, 2176:4224])
            nc.gpsimd.dma_start(out=o_sb[:, 2048:4096], in_=o_d[:, 2048:4096])
            nc.gpsimd.dma_start(out=stk[:, 4224:8320], in_=p03_d[:, 4224:8320])
            nc.gpsimd.dma_start(out=stk[:, 8320:12416], in_=p47_d[:, 0:4096])
            nc.gpsimd.dma_start(out=stk[:, 12416:16512], in_=p47_d[:, 4096:8192])

            # --- steady state ---
            # Two-way split of the psum+o -> fp16 elementwise stage:
            #   [0:xv]     DVE fused tensor_tensor from PSUM
            #   [xv:CHUNK] Act copy psum->fp16 tmp, Pool tensor_tensor add
            # xv=1300 balances DVE ((120+xv)/0.96 ns) against Pool
            # (144+1.79*(2048-xv) ns) at a ~1.65us/chunk cadence.
            # Out staging in 4-chunk mega tiles [128, 8192], written as two
            # [128, 4096] half-descriptors on both HWDGE queues at once
            # (8 KB per-partition packets; 4 KB packets cap near
            # ~140 GB/s/queue). The final mega drains in quarters.
            xv = int(os.environ.get("BT_XV", "1300"))
            MEGA = 4  # chunks per out staging tile
            MC = MEGA * CHUNK
            add = mybir.AluOpType.add
            # processing order: evens then odds within each block of 8, so
            # o's second half (obase=2048) is first needed at position 4
            order = []
            for blk in range(0, 32, 8):
                order += list(range(blk, blk + 8, 2)) + list(range(blk + 1, blk + 8, 2))
            ot = None
            pend = None
            for pos, ci in enumerate(order):
                c0 = ci * CHUNK
                po0 = pos * CHUNK  # out_d column base for this position
                mi = pos % MEGA
                pbase = c0 // 4
                obase = c0 % ocols
                ps = psum_pool.tile([128, CHUNK], F32, name=f"ps{c0}", tag="ps")
                for g in range(4):
                    nc.tensor.matmul(ps[:, 512 * g:512 * (g + 1)],
                                     wc_sb[32 * g:32 * (g + 1), 0:128],
                                     stk[32 * g:32 * (g + 1),
                                         128 + pbase:128 + pbase + 512],
                                     start=True, stop=True,
                                     tile_position=(32 * g, 0))
                if mi == 0:
                    ot = outc_pool.tile([128, MC], F16, name=f"ot{c0}", tag="outc")
                    mega_base = po0
                mo = mi * CHUNK
                tmp = tmp_pool.tile([128, CHUNK - xv], F16, name=f"tm{c0}", tag="tmp")
                nc.scalar.copy(tmp[:], ps[:, xv:CHUNK])
                nc.vector.tensor_tensor(
                    ot[:, mo:mo + xv], ps[:, 0:xv], o_sb[:, obase:obase + xv], add)
                nc.gpsimd.tensor_tensor(
                    ot[:, mo + xv:mo + CHUNK], tmp[:],
                    o_sb[:, obase + xv:obase + CHUNK], add)
                # issue the previous mega as two half-descriptors, one per
                # HWDGE queue, delayed one chunk so the sem-wait on its last
                # chunk's adds never blocks either FIFO
                if pend is not None and (mi == 0 or pos == 31):
                    pc0, pot = pend
                    nc.sync.dma_start(out=out_d[:, pc0:pc0 + MC // 2],
                                      in_=pot[:, 0:MC // 2])
                    nc.scalar.dma_start(out=out_d[:, pc0 + MC // 2:pc0 + MC],
                                        in_=pot[:, MC // 2:])
                    pend = None
                if mi == MEGA - 1:
                    pend = (mega_base, ot)
            # final mega drains in quarters; the last quarter is further
            # split across both queues so the tail after the final adds is
            # a single [128, 1024] transfer per queue
            pc0, pot = pend
            for s in range(3):
                eng = nc.sync if s % 2 == 0 else nc.scalar
                eng.dma_start(out=out_d[:, pc0 + 2048 * s:pc0 + 2048 * (s + 1)],
                              in_=pot[:, 2048 * s:2048 * (s + 1)])
            nc.scalar.dma_start(out=out_d[:, pc0 + 6144:pc0 + 7168],
                                in_=pot[:, 6144:7168])
            nc.sync.dma_start(out=out_d[:, pc0 + 7168:pc0 + 8192],
                              in_=pot[:, 7168:8192])

    nc.compile()
    return nc


# ---------------- entry point ----------------

def kernel(**inputs):
    inputs = {k: np.asarray(v) for k, v in inputs.items()}
    o_L0, res_levels = _host_precompute(inputs)

    if "nc" not in _CACHE:
        _CACHE["nc"] = _build_nc()
    nc = _CACHE["nc"]

    nlev = T - L0
    K = 8 * nlev
    wc = np.concatenate(
        [_make_lhsT(np.asarray(inputs["out_layers"][L0 + li], np.float32))
         for li in range(nlev)], axis=0)
    wc_tall = np.tile(wc, (128 // K, 1)).astype(NP16)

    in_maps = []
    for q in range(8):
        rows = []
        for li in range(nlev):
            t8 = _pack_res8(np.ascontiguousarray(res_levels[li][q::8]))
            rows.append(np.tile(t8, (1, N // t8.shape[1])))
        stackq = np.concatenate(rows, axis=0)
        # interleave 512-col stripes into 4 row-group partition blocks:
        # tall[32g+k, 512t+s] = stack[k, 2048t+512g+s]
        tall = np.ascontiguousarray(
            stackq.reshape(K, N // 2048, 4, 512).transpose(2, 0, 1, 3)
            .reshape(4 * K, NT)).astype(NP16)
        # wc + pieces 0-3 pre-tiled in one image; pieces 4-7 need only
        # their L19/L20 rows from HBM (periodic L17/L18 rows are replicated
        # on device from pieces 0-3)
        p03 = np.ascontiguousarray(
            np.concatenate([wc_tall, tall[:, 0:8192]], axis=1))
        p47 = np.ascontiguousarray(tall[:, 8192:16384])
        m = {"p03": p03, "p47": p47,
             "o_init": _pack_o_mod4(o_L0[q::8]).astype(NP16)}
        in_maps.append(m)

    res = run_bass_kernel_spmd(nc, in_maps, list(range(8)))

    order = []
    for blk in range(0, 32, 8):
        order += list(range(blk, blk + 8, 2)) + list(range(blk + 1, blk + 8, 2))
    full = np.empty((2 ** T, 32), np.float32)
    for q in range(8):
        outq = res.results[q]["out"].astype(np.float32)
        nat = np.empty_like(outq)
        for pos, ci in enumerate(order):
            nat[:, 2048 * ci:2048 * (ci + 1)] = outq[:, 2048 * pos:2048 * (pos + 1)]
        full[q::8] = _unpack_o_mod4(nat)
    return full.reshape((2,) * T + (32,))
